# revision 10
# baseline (speedup 1.0000x reference)
"""Trainium2 Bass kernel for nn_AttentionBlock (B=8, C=128, W=2048).

Reference computation (per batch b):
    q = Wq @ x + bq ; k = Wk @ x + bk ; v = Wv @ x + bv        # [C, W]
    energy[i, j] = sum_c q[c, i] * k[c, j]                     # [W, W]
    attn = softmax(energy, axis=-1)
    out[c, i] = sum_j v[c, j] * attn[i, j]
    return gamma * out + x

Sharding: data-parallel over batch B across the 8 NeuronCores (1 batch each),
with the tiny projection weights replicated (no collectives).

Per-core algorithm (E^T layout: the softmax axis j sits on partitions):
    host precomputes A = Wk^T Wq, so energy^T = X^T (A X) + r 1^T + 1 c^T:
      the r term (r = X^T Wk^T bq, per-j = per-partition) folds into the
      G evacuation bias; the c term (per-i, free axis) scales softmax
      numerator and denominator identically, so it is DROPPED exactly.
    G  = A X + wr 1^T            [c, i]   (one 128x128 matmul vs two for Q,K)
    Vt_j = gamma * (X_j^T Wv^T)  [j, c]   (bv recovered in the epilogue:
                                           attn rows sum to 1)
    per half h (i in [h*1024, (h+1)*1024)), per key block j (16):
      ET(h,j) = X_j^T G_h        [j, i]  PSUM     (producer, PE)
      PT(h,j) = exp(ET)          [j, i]  SBUF bf16 (ACT; no max subtraction:
                                                    |energy| < 40, f32 exp ok)
    per i-block b (8 per half), consumers fused U+S in ONE matmul chain:
      UT(b)[i, c'] = sum_j PT_j[:, b]^T @ [Vt_j | ones]   [128, 129] PSUM
        (col 128 accumulates S = sum_j exp; the separate ones-matmul for the
         softmax denominator is gone entirely)
      ob = UT[:, :128] * (1/UT[:, 128])   (DVE recip + per-partition scale)
      t  = ob^T via identity matmul (PE, bf16 rate, f32 PSUM out)
      out[:, b] = t + (x + gamma*bv)      (DVE add, f32)

Engine assignment: ACT runs ONLY the 32 exps (the roofline: 2048^2 elems
at 1 elem/part/cycle @1.2GHz ~= 33us/body); PE ~70k cycles ~= 29us; DVE
does all evacuations + epilogue (~15us); Pool does xb precompute, the
vt ones-column memset and h1 output DMA dispatch.

Software pipeline: consumers lag producers by one half; a body's h1
consumers are emitted interleaved with the NEXT body's h0 producers so
ACT never waits at body boundaries. UNROLL bodies per hardware-loop
iteration; only the last body's h1 consumers drain at the seam.

Host-side prep (layout/packing only + tiny 128x128 GEMM):
    xh = bf16(x), xf = f32(x), mw = [A^T | Wv^T | I] bf16,
    wb = [wr | gamma*bv | gamma] f32.
"""

import numpy as np

B, C, W = 8, 128, 2048
NCORES = 8
JT = W // 128  # 16 key blocks
NH = 2  # query-axis halves
H = W // NH  # 1024
NB = H // 128  # 8 i-blocks per half
UNROLL = 8

_CACHE = {}


def _build_bass(reps=1, loop=False):
    from contextlib import ExitStack

    import concourse.mybir as mybir
    import concourse.tile as tile
    from concourse import bacc

    f32 = mybir.dt.float32
    bf16 = mybir.dt.bfloat16
    AF = mybir.ActivationFunctionType

    nc = bacc.Bacc(
        "TRN2",
        target_bir_lowering=False,
        debug=False,
        enable_asserts=False,
        num_devices=NCORES,
    )

    xf_d = nc.dram_tensor("xf", [C, W], f32, kind="ExternalInput").ap()
    xh_d = nc.dram_tensor("xh", [C, W], bf16, kind="ExternalInput").ap()
    mw_d = nc.dram_tensor("mw", [C, 3 * C], bf16, kind="ExternalInput").ap()
    wb_d = nc.dram_tensor("wb", [C, 3], f32, kind="ExternalInput").ap()
    out_d = nc.dram_tensor("out", [C, W], f32, kind="ExternalOutput").ap()

    with tile.TileContext(nc) as tc, ExitStack() as ctx:
        # input/body-state pools are double-buffered so body k+1's DMAs and
        # prologue overlap body k's tail
        mwp = ctx.enter_context(tc.tile_pool(name="mwp", bufs=2))
        xhp = ctx.enter_context(tc.tile_pool(name="xhp", bufs=2))
        xfp = ctx.enter_context(tc.tile_pool(name="xfp", bufs=2))
        gsp = ctx.enter_context(tc.tile_pool(name="gsp", bufs=2))
        vtp = ctx.enter_context(tc.tile_pool(name="vtp", bufs=2))
        xbp = ctx.enter_context(tc.tile_pool(name="xbp", bufs=2))
        ptp = ctx.enter_context(tc.tile_pool(name="ptp", bufs=34))
        rcp = ctx.enter_context(tc.tile_pool(name="rcp", bufs=4))
        obp = ctx.enter_context(tc.tile_pool(name="obp", bufs=4))
        outp = ctx.enter_context(tc.tile_pool(name="outp", bufs=4))
        # PSUM: et 2x2 banks + ut 2x1 + misc 2x1 = 8 banks.  The et pool is
        # PURE producer ETs so its 2-slot rotation couples only exp<->prod;
        # G chunks, V^T chunks and the transposes share the slack-rich misc
        # rotation (their readers are prompt DVE ops).
        etp = ctx.enter_context(tc.tile_pool(name="etp", bufs=2, space="PSUM"))
        utp = ctx.enter_context(tc.tile_pool(name="utp", bufs=2, space="PSUM"))
        miscp = ctx.enter_context(tc.tile_pool(name="miscp", bufs=2, space="PSUM"))

        def part1(it):
            """Input DMAs + G projection.  Emitted INSIDE the previous
            body's phase B so the G chain (et slot -> matmul -> DVE evac)
            completes under the exp stream and never gaps ACT at the
            body boundary."""
            st = {}
            mw = mwp.tile([C, 3 * C], bf16, tag="mw", name=f"mw{it}")
            nc.sync.dma_start(mw, mw_d)
            wb = mwp.tile([C, 3], f32, tag="wb", name=f"wb{it}")
            nc.sync.dma_start(wb, wb_d)
            xh = xhp.tile([C, W], bf16, tag="xh", name=f"xh{it}")
            for chk in range(2):
                sl = slice(chk * 1024, (chk + 1) * 1024)
                nc.sync.dma_start(xh[:, sl], xh_d[:, sl])
            xf = xfp.tile([C, W], f32, tag="xf", name=f"xf{it}")
            for chk in range(2):
                sl = slice(chk * 1024, (chk + 1) * 1024)
                nc.sync.dma_start(xf[:, sl], xf_d[:, sl])

            st["xh"], st["xf"] = xh, xf
            st["mwM"] = mw[:, 0:C]          # A^T = Wq^T Wk
            st["mwV"] = mw[:, C : 2 * C]    # Wv^T
            st["ident"] = mw[:, 2 * C : 3 * C]
            wr_col = wb[:, 0:1]             # Wk^T bq
            st["gbv"] = wb[:, 1:2]          # gamma * bv
            st["gam"] = wb[:, 2:3]          # gamma

            gs = gsp.tile([C, W], bf16, tag="gs", name=f"gs{it}")
            st["gs"] = gs
            for m in range(4):
                sl = slice(m * 512, (m + 1) * 512)
                gp = miscp.tile([C, 512], f32, tag="mp", name=f"gp{it}_{m}")
                nc.tensor.matmul(gp, st["mwM"], xh[:, sl], start=True, stop=True)
                nc.vector.tensor_scalar_add(gs[:, sl], gp, wr_col)
            return st

        def emit_body(it, st, prev_tail, next_part1):
            xh, xf, gs = st["xh"], st["xf"], st["gs"]
            mwV, ident = st["mwV"], st["ident"]
            gbv_col, gam_col = st["gbv"], st["gam"]

            # vt[:, j, 0:128] = gamma * V^T_j ; vt[:, j, 128] = 1.0
            vt = vtp.tile([C, JT, 129], bf16, tag="vt", name=f"vt{it}")

            def vtgroup(g):
                vp = miscp.tile([C, 512], f32, tag="mp", name=f"vp{it}_{g}")
                for t in range(4):
                    j = 4 * g + t
                    nc.tensor.matmul(
                        vp[:, t * 128 : (t + 1) * 128],
                        xh[:, j * 128 : (j + 1) * 128],
                        mwV,
                        start=True,
                        stop=True,
                    )
                nc.vector.tensor_scalar_mul(
                    vt[:, 4 * g : 4 * (g + 1), 0:128], vp, gam_col
                )

            xb = xbp.tile([C, W], f32, tag="xb", name=f"xb{it}")

            def memxb():
                nc.gpsimd.memset(vt[:, :, 128:129], 1.0)
                # xb = x + gamma*bv, off the critical path on Pool
                for hh in range(NH):
                    sl = slice(hh * H, (hh + 1) * H)
                    nc.gpsimd.tensor_scalar_add(xb[:, sl], xf[:, sl], gbv_col)

            pts = {}

            def prod(h, j):
                et = etp.tile([C, H], f32, tag="et", name=f"et{it}_{h}_{j}")
                for n in range(2):
                    nc.tensor.matmul(
                        et[:, n * 512 : (n + 1) * 512],
                        xh[:, j * 128 : (j + 1) * 128],
                        gs[:, h * H + n * 512 : h * H + (n + 1) * 512],
                        start=True,
                        stop=True,
                    )
                pt = ptp.tile([C, H], bf16, tag="pt", name=f"pt{it}_{h}_{j}")
                nc.scalar.activation(pt, et, AF.Exp)
                pts[(h, j)] = pt

            # consumer thunks for half h: t_k = accum(k) + finish(k-1),
            # t_8 = finish(7).  finish lags so the PE transpose never waits
            # on the DVE scale of the same block.
            def make_cons(h):
                uts = {}

                def accum(b):
                    ut = utp.tile([C, 129], f32, tag="ut", name=f"ut{it}_{h}_{b}")
                    for j in range(JT):
                        nc.tensor.matmul(
                            ut,
                            pts[(h, j)][:, b * 128 : (b + 1) * 128],
                            vt[:, j, :],
                            start=(j == 0),
                            stop=(j == JT - 1),
                        )
                    uts[b] = ut

                def finish(b):
                    ut = uts.pop(b)
                    rc = rcp.tile([C, 1], f32, tag="rc", name=f"rc{it}_{h}_{b}")
                    nc.vector.reciprocal_approx_fast(out=rc, in_=ut[:, 128:129])
                    ob = obp.tile([C, 128], bf16, tag="ob", name=f"ob{it}_{h}_{b}")
                    nc.vector.tensor_scalar_mul(ob, ut[:, 0:128], rc)
                    tp = miscp.tile([C, 128], f32, tag="mp", name=f"tp{it}_{h}_{b}")
                    nc.tensor.matmul(tp, ob, ident, start=True, stop=True)
                    ot = outp.tile([C, 128], f32, tag="ot", name=f"ot{it}_{h}_{b}")
                    pos = slice(h * H + b * 128, h * H + (b + 1) * 128)
                    nc.vector.tensor_add(ot, tp, xb[:, pos])
                    if h == 0:
                        nc.sync.dma_start(out_d[:, pos], ot)
                    else:
                        nc.gpsimd.dma_start(out_d[:, pos], ot)

                thunks = []
                for b in range(NB):
                    def t(b=b):
                        accum(b)
                        if b > 0:
                            finish(b - 1)
                    thunks.append(t)
                thunks.append(lambda: finish(NB - 1))
                return thunks

            def stagger(prods, extras):
                # distribute extras evenly between producers
                seq = []
                ne, np_ = len(extras), len(prods)
                ei = 0
                for i, p in enumerate(prods):
                    seq.append(p)
                    want = (i + 1) * ne // np_
                    while ei < want:
                        seq.append(extras[ei])
                        ei += 1
                seq.extend(extras[ei:])
                return seq

            # phase A: h0 producers x (VT first so their DVE evacs clear the
            # queue before the consumer fins arrive, then prev h1 consumers)
            extrasA = [lambda g=g: vtgroup(g) for g in range(4)]
            extrasA.append(memxb)
            extrasA.extend(prev_tail or [])
            for f in stagger([lambda h=0, j=j: prod(h, j) for j in range(JT)],
                             extrasA):
                f()
            # phase B: h1 producers x (this body's h0 consumers + next part1
            # early, so the G chain lands mid-phase with DVE slack)
            extrasB = make_cons(0)
            if next_part1 is not None:
                extrasB.insert(2, next_part1)
            for f in stagger([lambda h=1, j=j: prod(h, j) for j in range(JT)],
                             extrasB):
                f()
            return make_cons(1)

        def emit_chain(n_bodies):
            tail = None
            holder = {"st": part1(0)}
            for u in range(n_bodies):
                if u + 1 < n_bodies:
                    def np1(it2=(u + 1) % 2):
                        holder["st_next"] = part1(it2)
                else:
                    np1 = None
                st = holder["st"]
                tail = emit_body(u % 2, st, tail, np1)
                if np1 is not None:
                    holder["st"] = holder.pop("st_next")
            for f in tail:
                f()

        if loop and reps > 1:
            n_iters, rem = divmod(reps, UNROLL)
            with tc.For_i(0, n_iters, 1) as _i:
                emit_chain(UNROLL)
            if rem:
                emit_chain(rem)
        else:
            emit_chain(reps)

    nc.compile()
    return nc


def _get_bass(reps=1, loop=False):
    key = ("nc", reps, loop)
    if key not in _CACHE:
        _CACHE[key] = _build_bass(reps, loop)
    return _CACHE[key]


def _make_in_maps(inputs):
    import ml_dtypes

    f32 = np.float32
    f64 = np.float64
    bf16 = ml_dtypes.bfloat16
    wq = np.asarray(inputs["Wq"], dtype=f64)
    wk = np.asarray(inputs["Wk"], dtype=f64)
    wv = np.asarray(inputs["Wv"], dtype=f64)
    bq = np.asarray(inputs["bq"], dtype=f64).reshape(C)
    bv = np.asarray(inputs["bv"], dtype=f64).reshape(C, 1)
    gm = np.asarray(inputs["gamma"], dtype=f64).reshape(1, 1)

    mwM = (wq.T @ wk).astype(bf16)          # A^T, A = Wk^T Wq
    mwV = np.ascontiguousarray(wv.T).astype(bf16)
    ident = np.eye(C, dtype=bf16)
    mw = np.ascontiguousarray(np.concatenate([mwM, mwV, ident], axis=1))

    wr = (wk.T @ bq).reshape(C, 1)          # Wk^T bq
    gbv = gm * bv
    gamc = np.broadcast_to(gm, (C, 1))
    wb = np.ascontiguousarray(
        np.concatenate([wr, gbv, gamc], axis=1).astype(f32)
    )

    xin = np.asarray(inputs["x"], dtype=f32)
    return [
        {
            "xf": np.ascontiguousarray(xin[b]),
            "xh": np.ascontiguousarray(xin[b].astype(bf16)),
            "mw": mw,
            "wb": wb,
        }
        for b in range(B)
    ]


def kernel(x, Wq, bq, Wk, bk, Wv, bv, gamma):
    from concourse import bass_utils

    nc = _get_bass()
    in_maps = _make_in_maps(
        dict(x=x, Wq=Wq, bq=bq, Wk=Wk, bk=bk, Wv=Wv, bv=bv, gamma=gamma)
    )
    res = bass_utils.run_bass_kernel_spmd(nc, in_maps, core_ids=list(range(NCORES)))
    return np.stack([res.results[b]["out"] for b in range(B)], axis=0)


# revision 21
# speedup vs baseline: 1.3164x; 1.3164x over previous
"""Trainium2 Bass kernel for nn_AttentionBlock (B=8, C=128, W=2048).

Reference computation (per batch b):
    q = Wq @ x + bq ; k = Wk @ x + bk ; v = Wv @ x + bv        # [C, W]
    energy[i, j] = sum_c q[c, i] * k[c, j]                     # [W, W]
    attn = softmax(energy, axis=-1)
    out[c, i] = sum_j v[c, j] * attn[i, j]
    return gamma * out + x

Sharding: data-parallel over batch B across the 8 NeuronCores (1 batch
each), tiny weights replicated, no collectives.

Per-core algorithm (E^T layout: softmax axis j on partitions):
    host precomputes A = Wk^T Wq, so energy^T = X^T (A X) + r 1^T + 1 c^T:
      r (= X^T Wk^T bq, per-partition) folds into the G evacuation bias;
      c (per-i) scales softmax numerator and denominator identically and
      is dropped EXACTLY.
    G  = A X + wr 1^T            [c, i]  (replaces both Q and K projections)
    Vt_j = gamma * (X_j^T Wv^T)  [j, c]  (bv recovered via the residual
                                          input: attn rows sum to 1)
    per half h, per key block j:   ET(h,j) = X_j^T G_h   (PE -> PSUM)
                                   PT(h,j) = exp(ET)     (ACT -> SBUF bf16)
    per i-block b: UT(b)[i, c'] = sum_j PT_j[:, b]^T @ [Vt_j | ones]
      -- U and the softmax denominator S (col 128) in ONE accumulation.
    ob = UT[:, :128] * (1/S)  (DVE recip + scale)
    ot = ob + xt[b]           (xt = x^T + gamma*bv, shipped pre-swizzled)
    out^T rows -> DRAM [W, C]; the host transposes back to [C, W].

Engine budget per body (measured): ACT = 32 exps only (~34.8us, the
pacer); PE ~36us (producers 14.7 + accums 18.4 + G 0.9 + Vt 2.7); DVE
evacuations + recip/scale/residual ~19us; Pool: ones memset + h1 DMA.

Scheduling: in-order PE + 2-slot ET rotation means any PE burst longer
than ~2 exp periods stalls ACT.  So consumer accumulations are split
into 8-matmul half-chains and all non-producer PE work is interleaved
between producers by a per-slot budget.  Consumers lag producers by one
half; the next body's input DMAs + G chain are emitted inside phase A
so the exp stream never gaps at body boundaries.  G/V^T/transpose-free
PSUM: et 2x2 banks (pure producer rotation), ut 2x1, misc 2x1.
"""

import numpy as np

B, C, W = 8, 128, 2048
NCORES = 8
JT = W // 128  # 16 key blocks
NH = 2  # query-axis halves
H = W // NH  # 1024
NB = H // 128  # 8 i-blocks per half
UNROLL = 8

# PE-cost estimates (ns) for the budgeted interleave
SLOT_NS = 1086.0  # measured exp-slot cadence
PROD_NS = 460.0
ACCUM_HALF_NS = 575.0
VT_HALF_NS = 340.0
PART1_NS = 900.0

_CACHE = {}


def _build_bass(reps=1, loop=False, ablate=None):
    from contextlib import ExitStack

    import concourse.mybir as mybir
    import concourse.tile as tile
    from concourse import bacc

    f32 = mybir.dt.float32
    bf16 = mybir.dt.bfloat16
    AF = mybir.ActivationFunctionType

    nc = bacc.Bacc(
        "TRN2",
        target_bir_lowering=False,
        debug=False,
        enable_asserts=False,
        num_devices=NCORES,
    )

    # xt = x^T + gamma*bv, pre-swizzled to [p, b, c] with i = b*128 + p
    xt_d = nc.dram_tensor("xt", [C, JT * C], f32, kind="ExternalInput").ap()
    xh_d = nc.dram_tensor("xh", [C, W], bf16, kind="ExternalInput").ap()
    mw_d = nc.dram_tensor("mw", [C, 2 * C], bf16, kind="ExternalInput").ap()
    wb_d = nc.dram_tensor("wb", [C, 2], f32, kind="ExternalInput").ap()
    out_d = nc.dram_tensor("out", [W, C], f32, kind="ExternalOutput").ap()

    with tile.TileContext(nc) as tc, ExitStack() as ctx:
        mwp = ctx.enter_context(tc.tile_pool(name="mwp", bufs=2))
        xhp = ctx.enter_context(tc.tile_pool(name="xhp", bufs=2))
        xtp = ctx.enter_context(tc.tile_pool(name="xtp", bufs=2))
        gsp = ctx.enter_context(tc.tile_pool(name="gsp", bufs=2))
        vtp = ctx.enter_context(tc.tile_pool(name="vtp", bufs=2))
        ptp = ctx.enter_context(tc.tile_pool(name="ptp", bufs=34))
        rcp = ctx.enter_context(tc.tile_pool(name="rcp", bufs=4))
        obp = ctx.enter_context(tc.tile_pool(name="obp", bufs=4))
        outp = ctx.enter_context(tc.tile_pool(name="outp", bufs=4))
        etp = ctx.enter_context(tc.tile_pool(name="etp", bufs=2, space="PSUM"))
        utp = ctx.enter_context(tc.tile_pool(name="utp", bufs=2, space="PSUM"))
        miscp = ctx.enter_context(tc.tile_pool(name="miscp", bufs=2, space="PSUM"))

        def part1(it):
            """Input DMAs + G projection; emitted a half-phase ahead so the
            G chain completes under the exp stream."""
            st = {}
            mw = mwp.tile([C, 2 * C], bf16, tag="mw", name=f"mw{it}")
            nc.sync.dma_start(mw, mw_d)
            wb = mwp.tile([C, 2], f32, tag="wb", name=f"wb{it}")
            nc.sync.dma_start(wb, wb_d)
            xh = xhp.tile([C, W], bf16, tag="xh", name=f"xh{it}")
            for chk in range(2):
                sl = slice(chk * 1024, (chk + 1) * 1024)
                nc.sync.dma_start(xh[:, sl], xh_d[:, sl])
            xts = xtp.tile([C, JT, C], f32, tag="xt", name=f"xt{it}")
            xts_flat = xts  # [p, b, c]
            for chk in range(2):
                sl = slice(chk * 1024, (chk + 1) * 1024)
                nc.sync.dma_start(
                    xts_flat[:, chk * 8 : (chk + 1) * 8, :], xt_d[:, sl]
                )

            st["xh"], st["xt"] = xh, xts
            st["mwM"] = mw[:, 0:C]        # A^T = Wq^T Wk
            st["mwV"] = mw[:, C : 2 * C]  # Wv^T
            wr_col = wb[:, 0:1]           # Wk^T bq
            st["gam"] = wb[:, 1:2]        # gamma

            gs = gsp.tile([C, W], bf16, tag="gs", name=f"gs{it}")
            st["gs"] = gs
            for m in range(4):
                sl = slice(m * 512, (m + 1) * 512)
                gp = miscp.tile([C, 512], f32, tag="mp", name=f"gp{it}_{m}")
                nc.tensor.matmul(gp, st["mwM"], xh[:, sl], start=True, stop=True)
                nc.vector.tensor_scalar_add(gs[:, sl], gp, wr_col)
            return st

        def emit_body(it, st, prev_tail, next_part1):
            xh, xts, gs = st["xh"], st["xt"], st["gs"]
            mwV, gam_col = st["mwV"], st["gam"]

            # vt[:, j, 0:128] = gamma * V^T_j ; vt[:, j, 128] = 1.0
            vt = vtp.tile([C, JT, 129], bf16, tag="vt", name=f"vt{it}")
            vps = {}

            def vt_half(g, second):
                if not second:
                    vp = miscp.tile([C, 512], f32, tag="mp", name=f"vp{it}_{g}")
                    vps[g] = vp
                    for t in range(2):
                        j = 4 * g + t
                        nc.tensor.matmul(
                            vp[:, t * 128 : (t + 1) * 128],
                            xh[:, j * 128 : (j + 1) * 128],
                            mwV,
                            start=True,
                            stop=True,
                        )
                else:
                    vp = vps.pop(g)
                    for t in range(2, 4):
                        j = 4 * g + t
                        nc.tensor.matmul(
                            vp[:, t * 128 : (t + 1) * 128],
                            xh[:, j * 128 : (j + 1) * 128],
                            mwV,
                            start=True,
                            stop=True,
                        )
                    nc.vector.tensor_scalar_mul(
                        vt[:, 4 * g : 4 * (g + 1), 0:128], vp, gam_col
                    )

            def memones():
                nc.gpsimd.memset(vt[:, :, 128:129], 1.0)

            pts = {}

            def prod(h, j):
                et = etp.tile([C, H], f32, tag="et", name=f"et{it}_{h}_{j}")
                for n in range(2):
                    nc.tensor.matmul(
                        et[:, n * 512 : (n + 1) * 512],
                        xh[:, j * 128 : (j + 1) * 128],
                        gs[:, h * H + n * 512 : h * H + (n + 1) * 512],
                        start=True,
                        stop=True,
                    )
                pt = ptp.tile([C, H], bf16, tag="pt", name=f"pt{it}_{h}_{j}")
                nc.scalar.activation(pt, et, AF.Exp)
                pts[(h, j)] = pt

            def make_cons(h):
                """Scheduling units (pe_cost, fn) for half h's consumers:
                accumulations split into 8-matmul half-chains; the finish
                (recip/scale/residual) is PE-free."""
                uts = {}

                def accum_half(b, second):
                    if not second:
                        ut = utp.tile([C, 129], f32, tag="ut",
                                      name=f"ut{it}_{h}_{b}")
                        uts[b] = ut
                        jr = range(0, 8)
                    else:
                        ut = uts[b]
                        jr = range(8, JT)
                    for j in jr:
                        nc.tensor.matmul(
                            ut,
                            pts[(h, j)][:, b * 128 : (b + 1) * 128],
                            vt[:, j, :],
                            start=(j == 0),
                            stop=(j == JT - 1),
                        )

                def finish(b):
                    ut = uts.pop(b)
                    rc = rcp.tile([C, 1], f32, tag="rc", name=f"rc{it}_{h}_{b}")
                    nc.vector.reciprocal_approx_fast(out=rc, in_=ut[:, 128:129])
                    ob = obp.tile([C, 128], f32, tag="ob", name=f"ob{it}_{h}_{b}")
                    nc.vector.tensor_scalar_mul(ob, ut[:, 0:128], rc)
                    gb = h * NB + b
                    ot = outp.tile([C, 128], f32, tag="ot", name=f"ot{it}_{h}_{b}")
                    nc.vector.tensor_add(ot, ob, xts[:, gb, :])
                    pos = slice(gb * 128, (gb + 1) * 128)
                    if h == 0:
                        nc.sync.dma_start(out_d[pos, :], ot)
                    else:
                        nc.gpsimd.dma_start(out_d[pos, :], ot)

                units = []
                for b in range(NB):
                    units.append((ACCUM_HALF_NS,
                                  lambda b=b: accum_half(b, False)))
                    # lag the finish one half-chain so its DVE recip never
                    # head-blocks the queue waiting on the accum's stop
                    if b > 0:
                        units.append((0.0, lambda b=b: finish(b - 1)))
                    units.append((ACCUM_HALF_NS,
                                  lambda b=b: accum_half(b, True)))
                units.append((0.0, lambda: finish(NB - 1)))
                return units

            def sched(prods, units):
                """Greedy budgeted interleave: spend each producer slot's
                spare PE time (SLOT - PROD) on the next units in order."""
                seq = []
                budget = 0.0
                ui = 0
                for p in prods:
                    seq.append(p)
                    budget += SLOT_NS - PROD_NS
                    while ui < len(units) and units[ui][0] <= budget:
                        budget -= units[ui][0]
                        seq.append(units[ui][1])
                        ui += 1
                seq.extend(u[1] for u in units[ui:])
                return seq

            if ablate == "prodexp":
                for j in range(JT):
                    prod(0, j)
                if next_part1 is not None:
                    next_part1()
                for j in range(JT):
                    prod(1, j)
                ot = outp.tile([C, 128], f32, tag="ot", name=f"oa{it}")
                nc.vector.tensor_copy(ot, xts[:, 0, :])
                nc.sync.dma_start(out_d[0:128, :], ot)
                return []

            # phase A: h0 producers x (vt groups 0-1 + next part1 + prev
            # body's h1 consumers)
            unitsA = [
                (VT_HALF_NS, lambda: vt_half(0, False)),
                (VT_HALF_NS, lambda: vt_half(0, True)),
                (0.0, memones),
                (VT_HALF_NS, lambda: vt_half(1, False)),
                (VT_HALF_NS, lambda: vt_half(1, True)),
            ]
            if next_part1 is not None:
                unitsA.append((PART1_NS, next_part1))
            unitsA.extend(prev_tail or [])
            for f in sched([lambda h=0, j=j: prod(h, j) for j in range(JT)],
                           unitsA):
                f()
            # phase B: h1 producers x (vt groups 2-3 + this body's h0 cons)
            unitsB = [
                (VT_HALF_NS, lambda: vt_half(2, False)),
                (VT_HALF_NS, lambda: vt_half(2, True)),
                (VT_HALF_NS, lambda: vt_half(3, False)),
                (VT_HALF_NS, lambda: vt_half(3, True)),
            ]
            unitsB.extend(make_cons(0))
            for f in sched([lambda h=1, j=j: prod(h, j) for j in range(JT)],
                           unitsB):
                f()
            return make_cons(1)

        def emit_chain(n_bodies):
            tail = None
            holder = {"st": part1(0)}
            for u in range(n_bodies):
                if u + 1 < n_bodies:
                    def np1(it2=(u + 1) % 2):
                        holder["st_next"] = part1(it2)
                else:
                    np1 = None
                st = holder["st"]
                tail = emit_body(u % 2, st, tail, np1)
                if np1 is not None:
                    holder["st"] = holder.pop("st_next")
            for _cost, f in tail:
                f()

        if loop and reps > 1:
            n_iters, rem = divmod(reps, UNROLL)
            with tc.For_i(0, n_iters, 1) as _i:
                emit_chain(UNROLL)
            if rem:
                emit_chain(rem)
        else:
            emit_chain(reps)

    nc.compile()
    return nc


def _get_bass(reps=1, loop=False):
    key = ("nc", reps, loop)
    if key not in _CACHE:
        _CACHE[key] = _build_bass(reps, loop)
    return _CACHE[key]


def _make_in_maps(inputs):
    import ml_dtypes

    f32 = np.float32
    f64 = np.float64
    bf16 = ml_dtypes.bfloat16
    wq = np.asarray(inputs["Wq"], dtype=f64)
    wk = np.asarray(inputs["Wk"], dtype=f64)
    wv = np.asarray(inputs["Wv"], dtype=f64)
    bq = np.asarray(inputs["bq"], dtype=f64).reshape(C)
    bv = np.asarray(inputs["bv"], dtype=f64).reshape(C)
    gm = float(np.asarray(inputs["gamma"], dtype=f64).reshape(()))

    mwM = (wq.T @ wk).astype(bf16)          # A^T, A = Wk^T Wq
    mwV = np.ascontiguousarray(wv.T).astype(bf16)
    mw = np.ascontiguousarray(np.concatenate([mwM, mwV], axis=1))

    wr = (wk.T @ bq).reshape(C, 1)          # Wk^T bq
    gamc = np.full((C, 1), gm, dtype=f64)
    wb = np.ascontiguousarray(np.concatenate([wr, gamc], axis=1).astype(f32))

    xin = np.asarray(inputs["x"], dtype=f32)
    maps = []
    for b in range(B):
        xb = xin[b]
        # xt[p, blk, c] = x[c, blk*128+p] + gamma*bv[c]
        xt = (xb.T.astype(f64) + gm * bv[None, :]).astype(f32)
        xt = np.ascontiguousarray(
            xt.reshape(JT, C, C).transpose(1, 0, 2).reshape(C, JT * C)
        )
        maps.append(
            {
                "xt": xt,
                "xh": np.ascontiguousarray(xb.astype(bf16)),
                "mw": mw,
                "wb": wb,
            }
        )
    return maps


def kernel(x, Wq, bq, Wk, bk, Wv, bv, gamma):
    from concourse import bass_utils

    nc = _get_bass()
    in_maps = _make_in_maps(
        dict(x=x, Wq=Wq, bq=bq, Wk=Wk, bk=bk, Wv=Wv, bv=bv, gamma=gamma)
    )
    res = bass_utils.run_bass_kernel_spmd(nc, in_maps, core_ids=list(range(NCORES)))
    # device returns out^T [W, C]; host restores [C, W]
    return np.stack(
        [np.ascontiguousarray(res.results[b]["out"].T) for b in range(B)], axis=0
    )
